# revision 9
# baseline (speedup 1.0000x reference)
"""Trainium2 Bass kernel for ColumnMixedPrecisionLinear.

Computes out[b,s,o] = bias[o] + sum_i x_i[b,s,:] @ (wq_i * s_i[:,None]).T
where x is [4, 2048, 4096] fp32, wq_i are [4096, 1024] int8 slices of the
weight along the input dim, s_i are per-output-channel scales.

Strategy: data-parallel over tokens across 8 NeuronCores. Each core gets
1024 tokens of x (flattened [8192, 4096]) and the full weights, and computes
its [1024, 4096] output shard. No cross-device reduction needed.

Per-core kernel (all bf16 matmul, fp32 PSUM accumulate):
  1. x shard loaded with SWDGE fp32->bf16 cast DMA, then PE-transposed into
     xT [128d x 32dblk x 1024t] resident in SBUF.
  2. Per output chunk of 512 channels: wq tiles loaded with SWDGE
     int8->bf16 cast DMA, dequantized by per-partition scale on DVE,
     PE-transposed into WT [128d x 32dblk x 512o].
  3. Main matmul: psum[128t, 512o] = bias (ones-matmul trick) +
     sum_dblk xT[:,dblk,tblk].T @ WT[:,dblk,:]; drain via ACT/DVE copy, DMA out.

int8 weights are exact in bf16; x and w*s each round once to bf16
(~0.2% rel), output rel err ~3e-3.
"""

import numpy as np
import ml_dtypes

import concourse.bass as bass
import concourse.mybir as mybir
import concourse.tile as tile
from concourse import bacc
from concourse.bass_utils import run_bass_kernel_spmd
from concourse.masks import make_identity

P = 128
N_CORES = 8
B, S = 4, 2048
D_IN_SLICE = 1024
N_SLICES = 4
D = D_IN_SLICE * N_SLICES      # 4096 contraction dim
O = 4096                       # out features
T = (B * S) // N_CORES         # 1024 tokens per core

T_TILES = T // P               # 8
D_BLKS = D // P                # 32
D_BLKS_SLICE = D_IN_SLICE // P # 8
O_CHUNK = 512
O_CHUNKS = O // O_CHUNK        # 8
O_TILES_PER_CHUNK = O_CHUNK // P  # 4

BF16 = mybir.dt.bfloat16
FP32 = mybir.dt.float32
INT8 = mybir.dt.int8


def build_nc(reps: int = 1, parts: str = "full"):
    """v2: all transposes via DMA-xbar (DRAM scratch round trip); PE does
    only matmuls. nc.sync is reserved for xbar transposes; all other DMAs
    go via SWDGE (gpsimd), which also does the fp32->bf16 / int8->bf16 casts.

    reps>1 repeats the whole body (same inputs/outputs) for benchmarking:
    HW time ~= fixed + reps * kernel_time.
    parts: "full" | "mm" (matmuls+drains only, inputs memset) |
           "data" (casts/scales/transposes only, no matmuls).

    NOTE: the neuron NEFF cache is keyed on the HLO signature only, so any
    two builds with identical I/O signatures would collide in the cache. A
    dummy "vtag" output with variant-dependent shape makes each non-default
    variant's HLO unique. The production build (reps=1, full) has no vtag."""
    nc = bacc.Bacc(None, target_bir_lowering=False)

    part_id = {"full": 0, "mm": 1, "data": 2}[parts]
    if reps != 1 or parts != "full":
        nc.dram_tensor("vtag", [1, reps * 16 + part_id + 1], FP32,
                       kind="ExternalOutput")

    x_in = nc.dram_tensor("x", [T, D], FP32, kind="ExternalInput")
    wq_in = [
        nc.dram_tensor(f"wq{i}", [O, D_IN_SLICE], INT8, kind="ExternalInput")
        for i in range(N_SLICES)
    ]
    # host-rearranged scales: sc[i][p, g] = s_i[g*128 + p] for o-tile g
    sc_in = [
        nc.dram_tensor(f"sc{i}", [P, O // P], FP32, kind="ExternalInput")
        for i in range(N_SLICES)
    ]
    # host-prepared bias: biasb[k, o] = bf16(bias[o] / 128); ones-matmul adds it
    biasb_in = nc.dram_tensor("biasb", [P, O], BF16, kind="ExternalInput")
    out = nc.dram_tensor("out", [T, O], FP32, kind="ExternalOutput")

    D_COL = 512                    # x cast column-chunk width
    D_COLS = D // D_COL            # 8

    with tile.TileContext(nc) as tc:
        with (
            tc.tile_pool(name="const", bufs=1) as const,
            tc.tile_pool(name="xres", bufs=1) as xres,
            tc.tile_pool(name="wstage", bufs=6) as wstage,
            tc.tile_pool(name="wt", bufs=2) as wt_pool,
            tc.tile_pool(name="ostage", bufs=4) as ostage,
            tc.tile_pool(name="psm", bufs=4, space="PSUM") as psm,
            tc.tile_pool(name="dram", bufs=1, space="DRAM") as dram,
        ):
            ones = const.tile([P, P], BF16)
            nc.any.memset(ones[:], 1.0)

            biasb = const.tile([P, O], BF16)
            nc.gpsimd.dma_start(biasb[:], biasb_in[:])
            scs = []
            for i in range(N_SLICES):
                sct = const.tile([P, O // P], FP32, tag=f"sc{i}")
                nc.gpsimd.dma_start(sct[:], sc_in[i][:])
                scs.append(sct)

            # DRAM scratch (allocated once, reused across reps)
            xb_drams = []
            for dc in range(D_COLS):
                xbd = dram.tile([T, D_COL], BF16, tag=f"xbd{dc}", name=f"xbd{dc}")
                xb_drams.append(xbd)
            # scratch per (slice, ochunk): [512o, 1024d] bf16 for exact deps
            wdeq = [
                [
                    dram.tile([O_CHUNK, D_IN_SLICE], BF16, tag=f"wdeq_{i}_{c}", name=f"wdeq_{i}_{c}")
                    for c in range(O_CHUNKS)
                ]
                for i in range(N_SLICES)
            ]

            do_data = parts in ("full", "data")
            do_mm = parts in ("full", "mm")

            if not do_data:
                # matmul-only ablation: fill inputs once via memset
                xT_static = xres.tile([P, D_BLKS, T], BF16, name="xT_static")
                nc.any.memset(xT_static[:], 0.25)
                wt_static = wt_pool.tile([P, D_BLKS, O_CHUNK], BF16,
                                         name="wt_static")
                nc.any.memset(wt_static[:], 0.5)

            for _rep in range(reps):
                if do_data:
                    # ---- x: cast to bf16 in DRAM columns, xbar-load ----
                    xT = xres.tile([P, D_BLKS, T], BF16)  # [128d,32,1024t]
                    for dc in range(D_COLS):
                        # DRAM->DRAM cast fp32 -> bf16 (SWDGE)
                        nc.gpsimd.dma_start(
                            xb_drams[dc][:], x_in[:, dc * D_COL:(dc + 1) * D_COL]
                        )
                    for db in range(D_BLKS):
                        dc, sub = db // 4, db % 4
                        # [1024t, 128d] -> [128d, 1024t]
                        nc.sync.dma_start_transpose(
                            xT[:, db, :],
                            xb_drams[dc][:, sub * P:(sub + 1) * P],
                        )

                    # ---- W: cast+scale per o-tile, store to scratch ----
                    for c in range(O_CHUNKS):
                        for i in range(N_SLICES):
                            for ot in range(O_TILES_PER_CHUNK):
                                g = c * O_TILES_PER_CHUNK + ot
                                wb = wstage.tile([P, D_IN_SLICE], BF16,
                                                 tag="wb")
                                nc.gpsimd.dma_start(
                                    wb[:], wq_in[i][g * P:(g + 1) * P, :]
                                )
                                nc.vector.tensor_scalar_mul(
                                    wb[:], wb[:], scs[i][:, g:g + 1]
                                )
                                nc.gpsimd.dma_start(
                                    wdeq[i][c][ot * P:(ot + 1) * P, :], wb[:]
                                )
                else:
                    xT = xT_static

                # ---- main loop per ochunk: xbar-load WT, matmuls ----
                for c in range(O_CHUNKS):
                    if do_data:
                        wt = wt_pool.tile([P, D_BLKS, O_CHUNK], BF16,
                                          tag="wt")
                        for i in range(N_SLICES):
                            for db in range(D_BLKS_SLICE):
                                # [512o, 128d] -> [128d, 512o]
                                nc.sync.dma_start_transpose(
                                    wt[:, i * D_BLKS_SLICE + db, :],
                                    wdeq[i][c][:, db * P:(db + 1) * P],
                                )
                    else:
                        wt = wt_static

                    if not do_mm:
                        # keep the data path live (defeat DCE): consume a
                        # sliver of wt/xT into the output
                        ob = ostage.tile([P, O_CHUNK], FP32, tag="ob")
                        nc.any.tensor_copy(ob[:, 0:P], wt[:, 0, 0:P])
                        nc.any.tensor_copy(ob[:, P:2 * P], xT[:, c, 0:P])
                        nc.gpsimd.dma_start(
                            out[0:P, c * O_CHUNK:c * O_CHUNK + 2 * P],
                            ob[:, 0:2 * P],
                        )
                        continue
                    for j in range(T_TILES):
                        ps = psm.tile([P, O_CHUNK], FP32, tag="ps")
                        # bias: sum_k ones[k,t] * (bias[o]/128) = bias[o]
                        nc.tensor.matmul(
                            ps[:], ones[:],
                            biasb[:, c * O_CHUNK:(c + 1) * O_CHUNK],
                            start=True, stop=False,
                        )
                        for db in range(D_BLKS):
                            nc.tensor.matmul(
                                ps[:],
                                xT[:, db, j * P:(j + 1) * P],
                                wt[:, db, :],
                                start=False, stop=(db == D_BLKS - 1),
                            )
                        ob = ostage.tile([P, O_CHUNK], FP32, tag="ob")
                        nc.any.tensor_copy(ob[:], ps[:])
                        nc.gpsimd.dma_start(
                            out[j * P:(j + 1) * P,
                                c * O_CHUNK:(c + 1) * O_CHUNK],
                            ob[:],
                        )
    nc.compile()
    return nc


_NC_CACHE = None


def _get_nc():
    global _NC_CACHE
    if _NC_CACHE is None:
        _NC_CACHE = build_nc()
    return _NC_CACHE


def _prep_inputs(x, wqs, ss, bias):
    xf = np.ascontiguousarray(np.asarray(x, dtype=np.float32).reshape(B * S, D))
    wqs = [np.ascontiguousarray(np.asarray(w).astype(np.int8)) for w in wqs]
    scs = [
        np.ascontiguousarray(np.asarray(s, dtype=np.float32).reshape(O // P, P).T)
        for s in ss
    ]
    biasb = np.ascontiguousarray(
        np.broadcast_to(
            (np.asarray(bias, dtype=np.float32) / 128.0).astype(ml_dtypes.bfloat16),
            (P, O),
        )
    )
    in_maps = []
    for c in range(N_CORES):
        m = {"x": xf[c * T:(c + 1) * T], "biasb": biasb}
        for i in range(N_SLICES):
            m[f"wq{i}"] = wqs[i]
            m[f"sc{i}"] = scs[i]
        in_maps.append(m)
    return in_maps


def run_on_hw(x, wqs, ss, bias, **spmd_kwargs):
    """Run and return (out_full [B,S,O] fp32, BassKernelResults)."""
    nc = _get_nc()
    in_maps = _prep_inputs(x, wqs, ss, bias)
    res = run_bass_kernel_spmd(nc, in_maps, core_ids=list(range(N_CORES)),
                               **spmd_kwargs)
    out = np.concatenate([r["out"] for r in res.results], axis=0)
    return np.ascontiguousarray(out.reshape(B, S, O).astype(np.float32)), res


def kernel(x, wq0, s0, wq1, s1, wq2, s2, wq3, s3, bias):
    out, _ = run_on_hw(x, [wq0, wq1, wq2, wq3], [s0, s1, s2, s3], bias)
    return out


# revision 12
# speedup vs baseline: 4.0601x; 4.0601x over previous
"""Trainium2 Bass kernel for ColumnMixedPrecisionLinear.

Computes out[b,s,o] = bias[o] + sum_i x_i[b,s,:] @ (wq_i * s_i[:,None]).T
where x is [4, 2048, 4096] fp32, wq_i are [4096, 1024] int8 slices of the
weight along the input dim, s_i are per-output-channel scales.

Strategy: data-parallel over tokens across 8 NeuronCores. Each core gets
1024 tokens of x (flattened [8192, 4096]) and the full weights, and computes
its [1024, 4096] output shard. No cross-device reduction needed.

Per-core kernel (all bf16 matmul, fp32 PSUM accumulate):
  1. x shard loaded with SWDGE fp32->bf16 cast DMA, then PE-transposed into
     xT [128d x 32dblk x 1024t] resident in SBUF.
  2. Per output chunk of 512 channels: wq tiles loaded with SWDGE
     int8->bf16 cast DMA, dequantized by per-partition scale on DVE,
     PE-transposed into WT [128d x 32dblk x 512o].
  3. Main matmul: psum[128t, 512o] = bias (ones-matmul trick) +
     sum_dblk xT[:,dblk,tblk].T @ WT[:,dblk,:]; drain via ACT/DVE copy, DMA out.

int8 weights are exact in bf16; x and w*s each round once to bf16
(~0.2% rel), output rel err ~3e-3.
"""

import numpy as np
import ml_dtypes

import concourse.bass as bass
import concourse.mybir as mybir
import concourse.tile as tile
from concourse import bacc
from concourse.bass_utils import run_bass_kernel_spmd
from concourse.masks import make_identity

P = 128
N_CORES = 8
B, S = 4, 2048
D_IN_SLICE = 1024
N_SLICES = 4
D = D_IN_SLICE * N_SLICES      # 4096 contraction dim
O = 4096                       # out features
T = (B * S) // N_CORES         # 1024 tokens per core

T_TILES = T // P               # 8
D_BLKS = D // P                # 32
D_BLKS_SLICE = D_IN_SLICE // P # 8
O_CHUNK = 512
O_CHUNKS = O // O_CHUNK        # 8
O_TILES_PER_CHUNK = O_CHUNK // P  # 4

BF16 = mybir.dt.bfloat16
FP32 = mybir.dt.float32
INT8 = mybir.dt.int8


def build_nc(reps: int = 1, parts: str = "full"):
    """v2: all transposes via DMA-xbar (DRAM scratch round trip); PE does
    only matmuls. nc.sync is reserved for xbar transposes; all other DMAs
    go via SWDGE (gpsimd), which also does the fp32->bf16 / int8->bf16 casts.

    reps>1 repeats the whole body (same inputs/outputs) for benchmarking:
    HW time ~= fixed + reps * kernel_time.
    parts: "full" | "mm" (matmuls+drains only, inputs memset) |
           "data" (casts/scales/transposes only, no matmuls).

    NOTE: the neuron NEFF cache is keyed on the HLO signature only, so any
    two builds with identical I/O signatures would collide in the cache. A
    dummy "vtag" output with variant-dependent shape makes each non-default
    variant's HLO unique. The production build (reps=1, full) has no vtag."""
    nc = bacc.Bacc(None, target_bir_lowering=False)

    part_id = {"full": 0, "mm": 1, "data": 2}[parts]
    if reps != 1 or parts != "full":
        nc.dram_tensor("vtag", [1, reps * 16 + part_id + 1], FP32,
                       kind="ExternalOutput")

    x_in = nc.dram_tensor("x", [T, D], FP32, kind="ExternalInput")
    wq_in = [
        nc.dram_tensor(f"wq{i}", [O, D_IN_SLICE], INT8, kind="ExternalInput")
        for i in range(N_SLICES)
    ]
    # host-rearranged scales: sc[i][p, g] = s_i[g*128 + p] for o-tile g
    sc_in = [
        nc.dram_tensor(f"sc{i}", [P, O // P], FP32, kind="ExternalInput")
        for i in range(N_SLICES)
    ]
    # host-prepared bias: biasb[k, o] = bf16(bias[o] / 128); ones-matmul adds it
    biasb_in = nc.dram_tensor("biasb", [P, O], BF16, kind="ExternalInput")
    out = nc.dram_tensor("out", [T, O], FP32, kind="ExternalOutput")

    D_COL = 1024                   # x cast column-chunk width
    D_COLS = D // D_COL            # 4
    D_BLKS_COL = D_COL // P        # 8 d-blocks per column chunk

    with tile.TileContext(nc) as tc:
        with (
            tc.tile_pool(name="const", bufs=1) as const,
            tc.tile_pool(name="xres", bufs=1) as xres,
            tc.tile_pool(name="wstage", bufs=2) as wstage,
            tc.tile_pool(name="wt", bufs=2) as wt_pool,
            tc.tile_pool(name="ostage", bufs=2) as ostage,
            tc.tile_pool(name="psm", bufs=4, space="PSUM") as psm,
            tc.tile_pool(name="dram", bufs=1, space="DRAM") as dram,
        ):
            ones = const.tile([P, P], BF16)
            nc.any.memset(ones[:], 1.0)

            biasb = const.tile([P, O], BF16)
            nc.gpsimd.dma_start(biasb[:], biasb_in[:])
            scs = []
            for i in range(N_SLICES):
                sct = const.tile([P, O // P], FP32, tag=f"sc{i}")
                nc.gpsimd.dma_start(sct[:], sc_in[i][:])
                scs.append(sct)

            # DRAM scratch (allocated once, reused across reps)
            xb_drams = []
            for dc in range(D_COLS):
                xbd = dram.tile([T, D_COL], BF16, tag=f"xbd{dc}", name=f"xbd{dc}")
                xb_drams.append(xbd)
            # scratch per (slice, ochunk): [512o, 1024d] bf16 for exact deps
            wdeq = [
                [
                    dram.tile([O_CHUNK, D_IN_SLICE], BF16, tag=f"wdeq_{i}_{c}", name=f"wdeq_{i}_{c}")
                    for c in range(O_CHUNKS)
                ]
                for i in range(N_SLICES)
            ]

            do_data = parts in ("full", "data")
            do_mm = parts in ("full", "mm")

            if not do_data:
                # matmul-only ablation: fill inputs once via memset
                xT_static = xres.tile([P, D_BLKS, T], BF16, name="xT_static")
                nc.any.memset(xT_static[:], 0.25)
                wt_static = wt_pool.tile([P, D_BLKS, O_CHUNK], BF16,
                                         name="wt_static")
                nc.any.memset(wt_static[:], 0.5)

            for _rep in range(reps):
                if do_data:
                    # ---- x: cast to bf16 in DRAM columns, xbar-load ----
                    # 3D xbar dst [128, J, R]: (p, j) holds src column
                    # c = j*128 + p (verified on HW) == our d-block layout.
                    xT = xres.tile([P, D_BLKS, T], BF16)  # [128d,32,1024t]
                    for dc in range(D_COLS):
                        # DRAM->DRAM cast fp32 -> bf16 (SWDGE)
                        nc.gpsimd.dma_start(
                            xb_drams[dc][:], x_in[:, dc * D_COL:(dc + 1) * D_COL]
                        )
                        # [1024t, 1024d] -> [128, 8, 1024t] in one xbar DMA
                        nc.sync.dma_start_transpose(
                            xT[:, dc * D_BLKS_COL:(dc + 1) * D_BLKS_COL, :],
                            xb_drams[dc][:],
                        )

                    # ---- W: cast+scale per (slice, chunk), store ----
                    for c in range(O_CHUNKS):
                        for i in range(N_SLICES):
                            # [512o, 1024d] int8 -> bf16 [128, 4sub, 1024]
                            # with o = sub*128 + p
                            wb = wstage.tile(
                                [P, O_TILES_PER_CHUNK, D_IN_SLICE], BF16,
                                tag="wb",
                            )
                            nc.gpsimd.dma_start(
                                wb[:],
                                wq_in[i][c * O_CHUNK:(c + 1) * O_CHUNK, :]
                                .rearrange("(sub p) d -> p sub d", p=P),
                            )
                            # scale: s[g*128+p] = scs[p, g], g = c*4 + sub
                            nc.vector.tensor_tensor(
                                wb[:], wb[:],
                                scs[i][:, c * O_TILES_PER_CHUNK:
                                       (c + 1) * O_TILES_PER_CHUNK, None]
                                .to_broadcast(
                                    (P, O_TILES_PER_CHUNK, D_IN_SLICE)),
                                mybir.AluOpType.mult,
                            )
                            nc.gpsimd.dma_start(
                                wdeq[i][c][:]
                                .rearrange("(sub p) d -> p sub d", p=P),
                                wb[:],
                            )
                else:
                    xT = xT_static

                # ---- main loop per ochunk: xbar-load WT, matmuls ----
                for c in range(O_CHUNKS):
                    if do_data:
                        wt = wt_pool.tile([P, D_BLKS, O_CHUNK], BF16,
                                          tag="wt")
                        for i in range(N_SLICES):
                            # [512o, 1024d] -> [128, 8, 512o] in one xbar DMA
                            nc.sync.dma_start_transpose(
                                wt[:, i * D_BLKS_SLICE:
                                   (i + 1) * D_BLKS_SLICE, :],
                                wdeq[i][c][:],
                            )
                    else:
                        wt = wt_static

                    if not do_mm:
                        # keep the data path live (defeat DCE): consume a
                        # sliver of wt/xT into the output
                        ob0 = ostage.tile([P, 2 * P], FP32, tag="ob0")
                        nc.any.tensor_copy(ob0[:, 0:P], wt[:, 0, 0:P])
                        nc.any.tensor_copy(ob0[:, P:2 * P], xT[:, c, 0:P])
                        nc.gpsimd.dma_start(
                            out[0:P, c * O_CHUNK:c * O_CHUNK + 2 * P],
                            ob0[:],
                        )
                        continue
                    ob = ostage.tile([P, T_TILES, O_CHUNK], FP32, tag="ob")
                    for j in range(T_TILES):
                        ps = psm.tile([P, O_CHUNK], FP32, tag="ps")
                        # bias: sum_k ones[k,t] * (bias[o]/128) = bias[o]
                        nc.tensor.matmul(
                            ps[:], ones[:],
                            biasb[:, c * O_CHUNK:(c + 1) * O_CHUNK],
                            start=True, stop=False,
                        )
                        for db in range(D_BLKS):
                            nc.tensor.matmul(
                                ps[:],
                                xT[:, db, j * P:(j + 1) * P],
                                wt[:, db, :],
                                start=False, stop=(db == D_BLKS - 1),
                            )
                        nc.any.tensor_copy(ob[:, j, :], ps[:])
                    # one 2 MiB store per chunk: rows t = j*128 + p
                    nc.gpsimd.dma_start(
                        out[:, c * O_CHUNK:(c + 1) * O_CHUNK]
                        .rearrange("(j p) o -> p j o", p=P),
                        ob[:],
                    )
    nc.compile()
    return nc


_NC_CACHE = None


def _get_nc():
    global _NC_CACHE
    if _NC_CACHE is None:
        _NC_CACHE = build_nc()
    return _NC_CACHE


def _prep_inputs(x, wqs, ss, bias):
    xf = np.ascontiguousarray(np.asarray(x, dtype=np.float32).reshape(B * S, D))
    wqs = [np.ascontiguousarray(np.asarray(w).astype(np.int8)) for w in wqs]
    scs = [
        np.ascontiguousarray(np.asarray(s, dtype=np.float32).reshape(O // P, P).T)
        for s in ss
    ]
    biasb = np.ascontiguousarray(
        np.broadcast_to(
            (np.asarray(bias, dtype=np.float32) / 128.0).astype(ml_dtypes.bfloat16),
            (P, O),
        )
    )
    in_maps = []
    for c in range(N_CORES):
        m = {"x": xf[c * T:(c + 1) * T], "biasb": biasb}
        for i in range(N_SLICES):
            m[f"wq{i}"] = wqs[i]
            m[f"sc{i}"] = scs[i]
        in_maps.append(m)
    return in_maps


def run_on_hw(x, wqs, ss, bias, **spmd_kwargs):
    """Run and return (out_full [B,S,O] fp32, BassKernelResults)."""
    nc = _get_nc()
    in_maps = _prep_inputs(x, wqs, ss, bias)
    res = run_bass_kernel_spmd(nc, in_maps, core_ids=list(range(N_CORES)),
                               **spmd_kwargs)
    out = np.concatenate([r["out"] for r in res.results], axis=0)
    return np.ascontiguousarray(out.reshape(B, S, O).astype(np.float32)), res


def kernel(x, wq0, s0, wq1, s1, wq2, s2, wq3, s3, bias):
    out, _ = run_on_hw(x, [wq0, wq1, wq2, wq3], [s0, s1, s2, s3], bias)
    return out
